# Initial kernel scaffold
#
"""ADMM LASSO solver on 8 TRN2 NeuronCores.

Data-parallel over the batch of observation columns (B=512 -> 64 per core).
Each core holds L = inv(H^T H + rho I) (replicated, resident in SBUF) and
runs 100 iterations of:
    right = Htx + rho*(v - u)
    s     = L @ right                  (PE, fp32, 16x16 128-tiles)
    v     = shrink(s + u, beta)        (DVE)
    u     = u + mu*(s - v)             (DVE)
    err_i = sum((H @ s - x)^2)         (PE fp32 + DVE reduce)
H^T (for the err GEMM) is streamed from DRAM each iteration.
L is computed on host exactly as the reference does (jax CPU fp32 inv) --
it is a precomputed constant in the original module.
Outputs: s [2048,512] fp32, errors [100] fp32 (partial sums reduced on host).
"""
import os
import sys

sys.path.insert(0, '/opt/trn_rl_repo')

import numpy as np

MU = 5e-05
LAMBDA_ = 12.5
MAX_ITER = int(os.environ.get("ADMM_ITERS", "100"))
RHO = 0.01
N, M, B = 1024, 2048, 512
NCORES = 8
BL = B // NCORES          # 64 columns per core
KT = M // 128             # 16 k-tiles
MT = M // 128             # 16 m-tiles for GEMM1
ET = N // 128             # 8 m-tiles for err GEMM
BETA = RHO / (2.0 * LAMBDA_)

_BUILt = {}


def _install_ntff_shim():
    """antenv.axon_hooks shim for NTFF profiling under axon (trace only)."""
    import contextlib, ctypes, types
    if "antenv.axon_hooks" in sys.modules:
        return
    import antenv
    so_path = "/opt/axon/libaxon_pjrt.so"
    lib = ctypes.CDLL(so_path)
    if not hasattr(lib, "axon_start_nrt_profile"):
        hook = None
    else:
        lib.axon_start_nrt_profile.argtypes = [ctypes.POINTER(ctypes.c_int64), ctypes.c_size_t]
        lib.axon_start_nrt_profile.restype = ctypes.c_int64
        lib.axon_stop_nrt_profile.argtypes = [ctypes.c_char_p]
        lib.axon_stop_nrt_profile.restype = ctypes.c_int64

        @contextlib.contextmanager
        def hook(output_dir, device_ids):
            import jax
            jax.devices()
            if device_ids:
                ids = (ctypes.c_int64 * len(device_ids))(*device_ids)
                rc = lib.axon_start_nrt_profile(ids, len(device_ids))
            else:
                rc = lib.axon_start_nrt_profile(None, 0)
            if rc != 0:
                raise RuntimeError(f"axon_start_nrt_profile rc={rc}")
            try:
                yield
            finally:
                n = lib.axon_stop_nrt_profile(str(output_dir).encode())
                if n <= 0:
                    print(f"ntff profile: rc={n} for {output_dir}", file=sys.stderr)

    mod = types.ModuleType("antenv.axon_hooks")
    holder = [hook]
    mod.set_axon_ntff_profile_hook = lambda h: holder.__setitem__(0, h)
    mod.get_axon_ntff_profile_hook = lambda: holder[0]
    sys.modules["antenv.axon_hooks"] = mod
    antenv.axon_hooks = mod


def _build():
    if "nc" in _BUILt:
        return _BUILt["nc"]
    import concourse.bass as bass
    import concourse.mybir as mybir
    import concourse.tile as tile
    from concourse import bacc

    f32 = mybir.dt.float32
    add = mybir.AluOpType.add
    sub = mybir.AluOpType.subtract
    mult = mybir.AluOpType.mult
    amax = mybir.AluOpType.max
    amin = mybir.AluOpType.min

    nc = bacc.Bacc(None)
    L_d = nc.declare_dram_parameter("L", [128, KT * M], f32, isOutput=False)
    Ht_d = nc.declare_dram_parameter("Ht", [128, KT * N], f32, isOutput=False)
    Hk_d = nc.declare_dram_parameter("Hk", [128, 16 * 1024], f32, isOutput=False)
    xl_d = nc.declare_dram_parameter("xl", [128, ET * BL], f32, isOutput=False)
    outs_d = nc.declare_dram_parameter("out_s", [128, MT * BL], f32, isOutput=True)
    oute_d = nc.declare_dram_parameter("out_e", [128, MAX_ITER], f32, isOutput=True)

    with tile.TileContext(nc) as tc:
        with (
            tc.tile_pool(name="weights", bufs=1) as wpool,
            tc.tile_pool(name="hstream", bufs=6) as hpool,
            tc.tile_pool(name="state", bufs=1) as spool,
            tc.tile_pool(name="tmp", bufs=3) as tpool,
            tc.tile_pool(name="psum", bufs=2, space=bass.MemorySpace.PSUM) as ppool,
        ):
            # resident weights: L (128KB/partition)
            l_sb = wpool.tile([128, KT * M], f32, tag="L")
            for k in range(KT):
                nc.sync.dma_start(out=l_sb[:, k * M:(k + 1) * M],
                                  in_=L_d[:, k * M:(k + 1) * M])
            x_sb = wpool.tile([128, ET * BL], f32, tag="x")
            nc.sync.dma_start(out=x_sb[:], in_=xl_d[:])

            # ---- setup: Htx = H^T @ x  (stream H half-tiles) ----
            htxp = ppool.tile([128, MT * BL], f32, tag="phtx")
            for k in range(8):
                for half in range(2):
                    hk = hpool.tile([128, 1024], f32, tag="hs")
                    nc.sync.dma_start(
                        out=hk[:],
                        in_=Hk_d[:, (k * 2 + half) * 1024:(k * 2 + half + 1) * 1024])
                    for m in range(8):
                        mg = half * 8 + m
                        nc.tensor.matmul(
                            htxp[:, mg * BL:(mg + 1) * BL],
                            hk[:, m * 128:(m + 1) * 128],
                            x_sb[:, k * BL:(k + 1) * BL],
                            start=(k == 0), stop=(k == 7))
            htx_sb = spool.tile([128, MT * BL], f32, tag="htx")
            nc.vector.tensor_copy(htx_sb[:], htxp[:])

            u_sb = spool.tile([128, MT * BL], f32, tag="u")
            v_sb = spool.tile([128, MT * BL], f32, tag="v")
            right_sb = spool.tile([128, MT * BL], f32, tag="right")
            s_sb = spool.tile([128, MT * BL], f32, tag="s")
            err_sb = spool.tile([128, MAX_ITER], f32, tag="err")
            nc.vector.memset(u_sb[:], 0.0)
            nc.vector.memset(v_sb[:], 0.0)

            for i in range(MAX_ITER):
                right = htx_sb if i == 0 else right_sb
                # ---- GEMM1: s = L @ right ----
                sp = ppool.tile([128, MT * BL], f32, tag="ps")
                for m in range(MT):
                    for k in range(KT):
                        nc.tensor.matmul(
                            sp[:, m * BL:(m + 1) * BL],
                            l_sb[:, k * M + m * 128: k * M + (m + 1) * 128],
                            right[:, k * BL:(k + 1) * BL],
                            start=(k == 0), stop=(k == KT - 1))
                nc.vector.tensor_copy(s_sb[:], sp[:])

                # ---- elementwise state update (DVE) ----
                z = tpool.tile([128, MT * BL], f32, tag="t0")
                nc.vector.tensor_add(z[:], s_sb[:], u_sb[:])
                a = tpool.tile([128, MT * BL], f32, tag="t1")
                nc.vector.tensor_scalar(a[:], z[:], BETA, 0.0, sub, amax)
                b = tpool.tile([128, MT * BL], f32, tag="t2")
                nc.vector.tensor_scalar(b[:], z[:], BETA, 0.0, add, amin)
                nc.vector.tensor_add(v_sb[:], a[:], b[:])
                d = tpool.tile([128, MT * BL], f32, tag="t0")
                nc.vector.tensor_sub(d[:], s_sb[:], v_sb[:])
                # u += mu * d
                nc.vector.scalar_tensor_tensor(u_sb[:], d[:], MU, u_sb[:], mult, add)
                w = tpool.tile([128, MT * BL], f32, tag="t1")
                nc.vector.tensor_sub(w[:], v_sb[:], u_sb[:])
                # right = Htx + rho * w
                nc.vector.scalar_tensor_tensor(right_sb[:], w[:], RHO, htx_sb[:], mult, add)

                # ---- err GEMM: Hs = H @ s  (stream H^T tiles) ----
                ep = ppool.tile([128, ET * BL], f32, tag="pe")
                for k in range(KT):
                    ht = hpool.tile([128, 1024], f32, tag="hs")
                    nc.sync.dma_start(out=ht[:], in_=Ht_d[:, k * N:(k + 1) * N])
                    for m in range(ET):
                        nc.tensor.matmul(
                            ep[:, m * BL:(m + 1) * BL],
                            ht[:, m * 128:(m + 1) * 128],
                            s_sb[:, k * BL:(k + 1) * BL],
                            start=(k == 0), stop=(k == KT - 1))
                e = tpool.tile([128, ET * BL], f32, tag="t2")
                nc.vector.tensor_sub(e[:], ep[:], x_sb[:])
                esq = tpool.tile([128, ET * BL], f32, tag="t0")
                nc.vector.tensor_tensor_reduce(
                    esq[:], e[:], e[:], 1.0, 0.0, mult, add, err_sb[:, i:i + 1])

            nc.sync.dma_start(out=outs_d[:], in_=s_sb[:])
            nc.sync.dma_start(out=oute_d[:], in_=err_sb[:])

    nc.compile()
    _BUILt["nc"] = nc
    return nc


def _compute_left_exact(H):
    """Replicate the reference's L = inv(H^T H + rho I) bit-for-bit:
    jax fp32 on CPU, same ops."""
    import jax
    import jax.numpy as jnp
    cpu = jax.devices("cpu")[0]
    with jax.default_device(cpu):
        Hj = jnp.asarray(np.asarray(H, np.float32))
        left = jnp.linalg.inv(Hj.T @ Hj + np.float32(RHO) * jnp.eye(M, dtype=Hj.dtype))
        return np.asarray(left)


def kernel(H, x):
    from concourse.bass_utils import run_bass_kernel_spmd

    H = np.asarray(H, np.float32)
    x = np.asarray(x, np.float32)
    trace = os.environ.get("ADMM_TRACE", "0") == "1"
    if trace:
        _install_ntff_shim()

    L = _compute_left_exact(H)

    # lhsT packing: tile (k,m)[p,q] must be Mat[m*128+q, k*128+p] -> pack Mat.T
    LT = np.ascontiguousarray(L.T)
    L_pack = np.ascontiguousarray(
        LT.reshape(KT, 128, M).transpose(1, 0, 2).reshape(128, KT * M))
    HT = np.ascontiguousarray(H.T)
    Ht_pack = np.ascontiguousarray(
        HT.reshape(KT, 128, N).transpose(1, 0, 2).reshape(128, KT * N))
    # Htx GEMM weights: tile (k,half)[p,j] = H[k*128+p, half*1024+j]
    Hk_pack = np.ascontiguousarray(
        H.reshape(8, 128, 2, 1024).transpose(1, 0, 2, 3).reshape(128, 16 * 1024))

    in_maps = []
    for c in range(NCORES):
        xl = np.ascontiguousarray(
            x[:, c * BL:(c + 1) * BL].reshape(ET, 128, BL).transpose(1, 0, 2)
            .reshape(128, ET * BL))
        in_maps.append({"L": L_pack, "Ht": Ht_pack, "Hk": Hk_pack, "xl": xl})

    nc = _build()
    res = run_bass_kernel_spmd(nc, in_maps, core_ids=list(range(NCORES)), trace=trace)
    if trace:
        print(f"HW exec time: {res.exec_time_ns} ns")
        _BUILt["last_exec_ns"] = res.exec_time_ns
        _BUILt["last_profile"] = res.profile_json

    s_parts = []
    err_acc = np.zeros(MAX_ITER, np.float64)
    for c in range(NCORES):
        out_s = res.results[c]["out_s"]
        s_parts.append(
            out_s.reshape(128, MT, BL).transpose(1, 0, 2).reshape(M, BL))
        err_acc += res.results[c]["out_e"][:, :MAX_ITER].astype(np.float64).sum(axis=0)
    s_full = np.concatenate(s_parts, axis=1)
    return s_full.astype(np.float32), err_acc.astype(np.float32)


# revision 44
# speedup vs baseline: 2.9633x; 2.9633x over previous
"""ADMM LASSO solver on 8 TRN2 NeuronCores.

Hybrid sharding: 4 data-parallel groups (pairs of cores) over the batch
columns (128 cols/pair) x 2-way model parallelism inside each pair over the
signal rows (1024 rows/core). Free dim 128 keeps the fp32 TensorEngine at
full rate (4 cyc/row; N=64 is weight-load-bound at 1/4 rate).

Per core, per iteration (stacked weights W = [L_half ; (H@L)_half], lhsT
[2048, 1536] resident in SBUF, 96KB/partition):
    right_local = Htx_half + rho*(v - u)           (DVE, own 1024 rows)
    pairwise AllGather right -> right_full [2048, 128]
    [s_half ; Hs_half] = W^T-stacked GEMM, split into local-k + remote-k
        partials so the AllGather overlaps the local-k matmuls
    v, u updates (shrinkage) on own rows          (DVE)
    err_i partial = sum((Hs_half - x_half)^2)     (DVE reduce)
L = inv(H^T H + rho I) is computed on host exactly as the reference does
(jax CPU fp32) -- a precomputed constant of the original module; H@L in fp32.
Outputs: s [2048,512] fp32, errors [100] fp32 (partials reduced on host).
"""
import os
import sys

sys.path.insert(0, '/opt/trn_rl_repo')

import numpy as np

MU = 5e-05
LAMBDA_ = 12.5
MAX_ITER = int(os.environ.get("ADMM_ITERS", "100"))
RHO = 0.01
N, M, B = 1024, 2048, 512
NCORES = 8
NPAIR = 4
BL = 128             # columns per pair
KT = M // 128        # 16 k-tiles over signal rows
MH = M // 2 // 128   # 8 m-tiles of own L-half rows
EH = N // 2 // 128   # 4 m-tiles of own HL-half rows
MT = MH + EH         # 12 stacked output m-tiles
BETA = RHO / (2.0 * LAMBDA_)

_BUILt = {}


def _install_ntff_shim():
    import contextlib, ctypes, types
    if "antenv.axon_hooks" in sys.modules:
        return
    import antenv
    so_path = "/opt/axon/libaxon_pjrt.so"
    lib = ctypes.CDLL(so_path)
    if not hasattr(lib, "axon_start_nrt_profile"):
        hook = None
    else:
        lib.axon_start_nrt_profile.argtypes = [ctypes.POINTER(ctypes.c_int64), ctypes.c_size_t]
        lib.axon_start_nrt_profile.restype = ctypes.c_int64
        lib.axon_stop_nrt_profile.argtypes = [ctypes.c_char_p]
        lib.axon_stop_nrt_profile.restype = ctypes.c_int64

        @contextlib.contextmanager
        def hook(output_dir, device_ids):
            import jax
            jax.devices()
            if device_ids:
                ids = (ctypes.c_int64 * len(device_ids))(*device_ids)
                rc = lib.axon_start_nrt_profile(ids, len(device_ids))
            else:
                rc = lib.axon_start_nrt_profile(None, 0)
            if rc != 0:
                raise RuntimeError(f"axon_start_nrt_profile rc={rc}")
            try:
                yield
            finally:
                n = lib.axon_stop_nrt_profile(str(output_dir).encode())
                if n <= 0:
                    print(f"ntff profile: rc={n} for {output_dir}", file=sys.stderr)

    mod = types.ModuleType("antenv.axon_hooks")
    holder = [hook]
    mod.set_axon_ntff_profile_hook = lambda h: holder.__setitem__(0, h)
    mod.get_axon_ntff_profile_hook = lambda: holder[0]
    sys.modules["antenv.axon_hooks"] = mod
    antenv.axon_hooks = mod


def _build():
    if "nc" in _BUILt:
        return _BUILt["nc"]
    import concourse.bass as bass
    import concourse.mybir as mybir
    import concourse.tile as tile
    from concourse import bacc

    f32 = mybir.dt.float32
    add = mybir.AluOpType.add
    sub = mybir.AluOpType.subtract
    mult = mybir.AluOpType.mult
    amax = mybir.AluOpType.max
    amin = mybir.AluOpType.min
    groups = [[2 * g, 2 * g + 1] for g in range(NPAIR)]

    nc = bacc.Bacc(None)
    W_d = nc.declare_dram_parameter("W", [128, KT * 1536], f32, isOutput=False)
    Hk_d = nc.declare_dram_parameter("Hk", [128, MH * 1024], f32, isOutput=False)
    xl_d = nc.declare_dram_parameter("xl", [128, 8 * BL], f32, isOutput=False)
    xe_d = nc.declare_dram_parameter("xe", [128, EH * BL], f32, isOutput=False)
    idx_d = nc.declare_dram_parameter("idx", [128, 1], mybir.dt.int32, isOutput=False)
    outs_d = nc.declare_dram_parameter("out_s", [128, MH * BL], f32, isOutput=True)
    oute_d = nc.declare_dram_parameter("out_e", [128, MAX_ITER], f32, isOutput=True)

    # AllGather concatenates the FLAT per-rank buffers: declare the output as
    # [2*128, cols] so rank-even lands in rows 0:128, rank-odd in 128:256.
    cc_in = [nc.dram_tensor(f"cc_in{j}", [128, MH * BL], f32) for j in range(2)]
    cc_out = [nc.dram_tensor(f"cc_out{j}", [2 * 128, MH * BL], f32) for j in range(4)]

    with tile.TileContext(nc) as tc:
        with (
            tc.tile_pool(name="weights", bufs=1) as wpool,
            tc.tile_pool(name="hstream", bufs=2) as hpool,
            tc.tile_pool(name="state", bufs=1) as spool,
            tc.tile_pool(name="rloc", bufs=2) as rpool,
            tc.tile_pool(name="tmp", bufs=1) as tpool,
            tc.tile_pool(name="psum", bufs=1, space=bass.MemorySpace.PSUM) as ppool,
        ):
            w_sb = wpool.tile([128, KT * 1536], f32, tag="W")
            x_sb = wpool.tile([128, 8 * BL], f32, tag="x")
            nc.sync.dma_start(out=x_sb[:], in_=xl_d[:])
            xe_sb = wpool.tile([128, EH * BL], f32, tag="xe")
            nc.sync.dma_start(out=xe_sb[:], in_=xe_d[:])
            idx_sb = wpool.tile([128, 1], mybir.dt.int32, tag="idx")
            nc.sync.dma_start(out=idx_sb[:], in_=idx_d[:])

            # ---- setup: Htx_half = (H^T x)[own 1024 rows] ----
            phtx = ppool.tile([128, MH * BL], f32, tag="pAL")
            for m in range(MH):
                hm = hpool.tile([128, 1024], f32, tag="hs")
                nc.sync.dma_start(out=hm[:], in_=Hk_d[:, m * 1024:(m + 1) * 1024])
                for k in range(8):
                    nc.tensor.matmul(
                        phtx[:, m * BL:(m + 1) * BL],
                        hm[:, k * 128:(k + 1) * 128],
                        x_sb[:, k * BL:(k + 1) * BL],
                        start=(k == 0), stop=(k == 7))
            htx_sb = spool.tile([128, MH * BL], f32, tag="htx")
            nc.vector.tensor_copy(htx_sb[:], phtx[:])

            # weight load AFTER the Htx-stream DMAs in the sync FIFO, split
            # per k-block so iteration 0 can start as blocks land
            for k in range(KT):
                nc.sync.dma_start(out=w_sb[:, k * 1536:(k + 1) * 1536],
                                  in_=W_d[:, k * 1536:(k + 1) * 1536])

            u_sb = spool.tile([128, MH * BL], f32, tag="u")
            v_sb = spool.tile([128, MH * BL], f32, tag="v")
            s_sb = spool.tile([128, MH * BL], f32, tag="s")
            hs_sb = spool.tile([128, EH * BL], f32, tag="hs_out")
            err_sb = spool.tile([128, MAX_ITER], f32, tag="err")
            nc.vector.memset(u_sb[:], 0.0)
            nc.vector.memset(v_sb[:], 0.0)

            # iteration 0's right_local is exactly Htx (rho*(0-0) added = 0)
            r0 = rpool.tile([128, MH * BL], f32, tag="rl")
            nc.vector.tensor_copy(r0[:], htx_sb[:])
            nc.sync.dma_start(out=cc_in[0][:], in_=r0[:])
            nc.gpsimd.collective_compute(
                "AllGather", mybir.AluOpType.bypass,
                ins=[cc_in[0][:]], outs=[cc_out[0][:]], replica_groups=groups)
            right_loc = r0

            for i in range(MAX_ITER):
                # Phase order A-L, A-HL, B-L, B-HL: the HL (err) matmuls are
                # PE filler that overlaps the AllGather latency and the DVE
                # chain, keeping the PE busy (and HAM warm) the whole
                # iteration. Local k-tiles are host-packed as the first 8
                # k-blocks of w_sb (per-core h-dependent), remote as the last 8.
                pAL = ppool.tile([128, MH * BL], f32, tag="pAL")
                for m in range(MH):
                    for k in range(8):
                        nc.tensor.matmul(
                            pAL[:, m * BL:(m + 1) * BL],
                            w_sb[:, k * 1536 + m * 128: k * 1536 + (m + 1) * 128],
                            right_loc[:, k * BL:(k + 1) * BL],
                            start=(k == 0), stop=(k == 7))
                pAH = ppool.tile([128, EH * BL], f32, tag="pAH")
                for m in range(EH):
                    for k in range(8):
                        nc.tensor.matmul(
                            pAH[:, m * BL:(m + 1) * BL],
                            w_sb[:, k * 1536 + 1024 + m * 128: k * 1536 + 1024 + (m + 1) * 128],
                            right_loc[:, k * BL:(k + 1) * BL],
                            start=(k == 0), stop=(k == 7))

                # Stage pA* to SBUF now (DVE is strict FIFO: these must be
                # emitted before the collective-gated tsel/rrem ops, and they
                # only depend on phase A). DVE reads at most one PSUM operand.
                sA = tpool.tile([128, MH * BL], f32, tag="t3")
                nc.vector.tensor_copy(sA[:], pAL[:])
                hsA = tpool.tile([128, EH * BL], f32, tag="t4")
                nc.vector.tensor_copy(hsA[:], pAH[:])

                # ---- remote half arrives via AllGather; gather the PEER's
                # shard rows directly with a per-core index tensor (rows
                # (1-h)*128..: graph stays SPMD-uniform, one DMA replaces the
                # lo/hi + masked-select chain) ----
                rrem = rpool.tile([128, MH * BL], f32, tag="rrem")
                nc.gpsimd.indirect_dma_start(
                    out=rrem[:], out_offset=None,
                    in_=cc_out[i % 4][:],
                    in_offset=bass.IndirectOffsetOnAxis(ap=idx_sb[:, :1], axis=0))
                pBL = ppool.tile([128, MH * BL], f32, tag="pBL")
                for m in range(MH):
                    for k in range(8):
                        nc.tensor.matmul(
                            pBL[:, m * BL:(m + 1) * BL],
                            w_sb[:, (8 + k) * 1536 + m * 128: (8 + k) * 1536 + (m + 1) * 128],
                            rrem[:, k * BL:(k + 1) * BL],
                            start=(k == 0), stop=(k == 7))
                pBH = ppool.tile([128, EH * BL], f32, tag="pBH")
                for m in range(EH):
                    for k in range(8):
                        nc.tensor.matmul(
                            pBH[:, m * BL:(m + 1) * BL],
                            w_sb[:, (8 + k) * 1536 + 1024 + m * 128: (8 + k) * 1536 + 1024 + (m + 1) * 128],
                            rrem[:, k * BL:(k + 1) * BL],
                            start=(k == 0), stop=(k == 7))

                # ---- state update on own rows (DVE), in two column-halves so
                # the half-chains pipeline and right_loc completes sooner ----
                right_loc = rpool.tile([128, MH * BL], f32, tag="rl")
                HB = MH * BL // 4
                z = tpool.tile([128, MH * BL], f32, tag="t0")
                a = tpool.tile([128, MH * BL], f32, tag="t1")
                b = tpool.tile([128, MH * BL], f32, tag="t2")
                for hf in range(4):
                    cs = slice(hf * HB, (hf + 1) * HB)
                    nc.vector.tensor_add(s_sb[:, cs], sA[:, cs], pBL[:, cs])
                    nc.vector.tensor_add(z[:, cs], s_sb[:, cs], u_sb[:, cs])
                    nc.vector.tensor_scalar(a[:, cs], z[:, cs], BETA, 0.0, sub, amax)
                    nc.vector.tensor_scalar(b[:, cs], z[:, cs], BETA, 0.0, add, amin)
                    nc.vector.tensor_add(v_sb[:, cs], a[:, cs], b[:, cs])
                    nc.vector.tensor_sub(z[:, cs], s_sb[:, cs], v_sb[:, cs])
                    nc.vector.scalar_tensor_tensor(u_sb[:, cs], z[:, cs], MU,
                                                   u_sb[:, cs], mult, add)
                    nc.vector.tensor_sub(a[:, cs], v_sb[:, cs], u_sb[:, cs])
                    nc.vector.scalar_tensor_tensor(right_loc[:, cs], a[:, cs], RHO,
                                                   htx_sb[:, cs], mult, add)
                nc.vector.tensor_add(hs_sb[:], hsA[:], pBH[:])

                if i + 1 < MAX_ITER:
                    nc.sync.dma_start(out=cc_in[(i + 1) % 2][:], in_=right_loc[:])
                    nc.gpsimd.collective_compute(
                        "AllGather", mybir.AluOpType.bypass,
                        ins=[cc_in[(i + 1) % 2][:]],
                        outs=[cc_out[(i + 1) % 4][:]], replica_groups=groups)

                # ---- err partial: sum((Hs_half - x_half)^2) ----
                e = tpool.tile([128, EH * BL], f32, tag="t1")
                nc.vector.tensor_sub(e[:], hs_sb[:], xe_sb[:])
                esq = tpool.tile([128, EH * BL], f32, tag="t2")
                nc.vector.tensor_mul(esq[:], e[:], e[:])
                nc.vector.reduce_sum(err_sb[:, i:i + 1], esq[:],
                                     axis=mybir.AxisListType.X)

            nc.sync.dma_start(out=outs_d[:], in_=s_sb[:])
            nc.sync.dma_start(out=oute_d[:], in_=err_sb[:])

    nc.compile()
    _BUILt["nc"] = nc
    return nc


def _compute_left_exact(H):
    """Replicate the reference's L = inv(H^T H + rho I) bit-for-bit:
    jax fp32 on CPU, same ops."""
    import jax
    import jax.numpy as jnp
    cpu = jax.devices("cpu")[0]
    with jax.default_device(cpu):
        Hj = jnp.asarray(np.asarray(H, np.float32))
        left = jnp.linalg.inv(Hj.T @ Hj + np.float32(RHO) * jnp.eye(M, dtype=Hj.dtype))
        return np.asarray(left)


def _pack_inputs(H, x, L):
    """Per-core input packing. Core c: pair g=c//2 (columns), half h=c%2 (rows).

    Weight pack per half h: 16 k-blocks of 1536 cols = [L-half 1024 | HL-half 512].
    k-blocks are ordered LOCAL-first: k-block j<8 -> global k-tile h*8+j (own
    right rows); j>=8 -> global k-tile (1-h)*8+(j-8) (remote rows)."""
    HL = (H.astype(np.float32) @ L.astype(np.float32)).astype(np.float32)
    LT = np.ascontiguousarray(L.T)    # LT[k*128+p, m*128+q] = L[m.., k..]
    HLT = np.ascontiguousarray(HL.T)

    W_packs = []
    for h in range(2):
        Wk = np.empty((128, KT * 1536), np.float32)
        for j in range(KT):
            kg = h * 8 + j if j < 8 else (1 - h) * 8 + (j - 8)
            lpart = LT[kg * 128:(kg + 1) * 128, h * 1024:(h + 1) * 1024]
            hlpart = HLT[kg * 128:(kg + 1) * 128, h * 512:(h + 1) * 512]
            Wk[:, j * 1536:j * 1536 + 1024] = lpart
            Wk[:, j * 1536 + 1024:(j + 1) * 1536] = hlpart
        W_packs.append(np.ascontiguousarray(Wk))

    # Htx setup weights per half: m-major blocks [128, 8*128], block (m,k):
    # Hk[p, m*1024 + k*128 + q] = H[k*128+p, h*1024 + m*128+q]
    Hk_packs = []
    for h in range(2):
        Hh = H[:, h * 1024:(h + 1) * 1024]
        Hk_packs.append(np.ascontiguousarray(
            Hh.reshape(8, 128, MH, 128).transpose(1, 2, 0, 3).reshape(128, MH * 1024)))

    # x per pair g: k-tiles over observation rows [1024] -> [128, 8*BL]
    xg = []
    for g in range(NPAIR):
        xc = x[:, g * BL:(g + 1) * BL]
        xg.append(np.ascontiguousarray(
            xc.reshape(8, 128, BL).transpose(1, 0, 2).reshape(128, 8 * BL)))
    # err x: rows [h*512:(h+1)*512] of pair g's columns
    xe = {}
    for g in range(NPAIR):
        for h in range(2):
            xc = x[h * 512:(h + 1) * 512, g * BL:(g + 1) * BL]
            xe[(g, h)] = np.ascontiguousarray(
                xc.reshape(EH, 128, BL).transpose(1, 0, 2).reshape(128, EH * BL))
    # row indices into cc_out selecting the PEER's shard: (1-h)*128 + p
    idx = []
    for h in range(2):
        idx.append((((1 - h) * 128) + np.arange(128, dtype=np.int32))
                   .reshape(128, 1))
    return W_packs, Hk_packs, xg, xe, idx


def kernel(H, x):
    from concourse.bass_utils import run_bass_kernel_spmd

    H = np.asarray(H, np.float32)
    x = np.asarray(x, np.float32)
    trace = os.environ.get("ADMM_TRACE", "0") == "1"
    if trace:
        _install_ntff_shim()

    L = _compute_left_exact(H)
    W_packs, Hk_packs, xg, xe, idx = _pack_inputs(H, x, L)

    in_maps = []
    for c in range(NCORES):
        g, h = divmod(c, 2)
        in_maps.append({"W": W_packs[h], "Hk": Hk_packs[h], "xl": xg[g],
                        "xe": xe[(g, h)], "idx": idx[h]})

    nc = _build()
    res = run_bass_kernel_spmd(nc, in_maps, core_ids=list(range(NCORES)), trace=trace)
    if trace:
        print(f"HW exec time: {res.exec_time_ns} ns")
        _BUILt["last_exec_ns"] = res.exec_time_ns
        _BUILt["last_profile"] = res.profile_json

    # reassemble: core (g,h) holds s rows [h*1024:(h+1)*1024], cols pair g
    s_full = np.empty((M, B), np.float32)
    err_acc = np.zeros(MAX_ITER, np.float64)
    for c in range(NCORES):
        g, h = divmod(c, 2)
        out_s = res.results[c]["out_s"]
        s_half = out_s.reshape(128, MH, BL).transpose(1, 0, 2).reshape(M // 2, BL)
        s_full[h * (M // 2):(h + 1) * (M // 2), g * BL:(g + 1) * BL] = s_half
        err_acc += res.results[c]["out_e"][:, :MAX_ITER].astype(np.float64).sum(axis=0)
    return s_full, err_acc.astype(np.float32)
